# revision 3
# baseline (speedup 1.0000x reference)
"""GroupLinear (MoE routing) Trainium2 kernel — bf16 W-stationary version.

Problem: x [8192, 1024] f32, indices [8192] int64 in [0,8),
W [8*2048, 1024] f32, b [8*2048] f32.
out[n] = x[n] @ W[g*2048:(g+1)*2048].T + b[g*2048:(g+1)*2048],  g = indices[n].

Strategy: expert-parallel across the 8 NeuronCores. Core g owns group g's
weight slice only and processes exactly the rows routed to group g (argsort
on host). Operands are cast to bf16 on host (rel-err ~3e-3, full PE rate,
half the HBM traffic of f32).

Device layout (W-stationary): the PE's stationary operand is a [128k x 128f]
W chunk; x rows are the MOVING operand, so per-core PE time is exactly
128 * c_pad cycles (no padding of rows to 128-blocks). PSUM tiles are
[128f x slab<=512 rows]; eviction adds the per-feature bias (it varies along
partitions) and casts to bf16, alternating between DVE (tensor_scalar_add)
and ACT (activation Identity+bias) so no single engine gates the tail.
Output leaves the device transposed [2048f x c_pad rows]; host transposes
back (host time is not graded).

Schedule: loads are split across BOTH HWDGE rings so the startup ramp is
fed at combined rate: sync gets [w0, x0, x2, x4, x6, w23, w67, w10-11,
w14-15], scalar gets [bias, w1, x1, x3, x5, x7, w45, w89, w12-13] then the
stores. fb0+fb1 are computed kc-interleaved, paced by x-chunk arrival; the
remaining 14 feature blocks are compute-bound and run slab-major so each
PSUM slab is evicted + stored while the next one computes. The last fb's
slab stores ride the (by then idle) sync ring. A short junk-matmul burst
lifts the PE HAM clock gate while the first loads are in flight.
"""

import os
import sys

sys.path.insert(0, "/opt/trn_rl_repo")

import numpy as np

import concourse.bass as bass
import concourse.bacc as bacc
import concourse.mybir as mybir
import concourse.tile as tile
from concourse.bass_utils import run_bass_kernel_spmd

N = 8192
IN_F = 1024
OUT_F = 2048
G = 8
NCORES = 8
P = 128
KC_N = IN_F // P      # 8 contraction chunks
FB_N = OUT_F // P     # 16 feature blocks (stationary tiles per kc)
SLAB_MAX = 512        # PSUM bank free size in fp32
CHUNK_MAX = 1512      # rows per chunk -> <=3 slabs -> <=3 psum banks per fb

N_WARM_LONG = 3       # junk matmuls N=512 (cold ~427ns each)
N_WARM_SHORT = 5      # junk matmuls N=128 (cold ~107ns each)

LAST_EXEC_NS = None
LAST_RESULTS = None

_nc_cache = {}


def _chunk_plan(c_pad):
    """Split c_pad rows into chunks of <=3 slabs, slab sizes multiple of 8."""
    chunks = []
    r0 = 0
    while r0 < c_pad:
        rem = c_pad - r0
        L = rem if rem <= 3 * SLAB_MAX else CHUNK_MAX
        n_s = -(-L // SLAB_MAX)
        base = -(-(-(-L // n_s)) // 8) * 8
        sizes = [base] * (n_s - 1)
        sizes.append(L - base * (n_s - 1))
        chunks.append((r0, sizes))
        r0 += L
    return chunks


def _build_nc(c_pad: int):
    """Per-core Bass program for c_pad routed rows (same program, all cores)."""
    assert c_pad % 8 == 0
    chunks = _chunk_plan(c_pad)
    f32 = mybir.dt.float32
    bf16 = mybir.dt.bfloat16

    nc = bacc.Bacc("TRN2", target_bir_lowering=False, debug=False)

    x_r = nc.dram_tensor("x_r", [P, KC_N * c_pad], bf16, kind="ExternalInput")
    w_r = nc.dram_tensor("w_r", [P, FB_N * IN_F], bf16, kind="ExternalInput")
    b_r = nc.dram_tensor("b_r", [P, FB_N], f32, kind="ExternalInput")
    out = nc.dram_tensor("out", [OUT_F, c_pad], bf16, kind="ExternalOutput")

    with tile.TileContext(nc) as tc:
        with (
            tc.tile_pool(name="wp", bufs=1) as wp,
            tc.tile_pool(name="xp", bufs=1) as xp,
            tc.tile_pool(name="bp", bufs=1) as bp,
            tc.tile_pool(name="op", bufs=3) as op,
            tc.tile_pool(name="pp", bufs=7, space="PSUM") as pp,
            tc.tile_pool(name="warm", bufs=1) as warmp,
            tc.tile_pool(name="warmps", bufs=1, space="PSUM") as warmpp,
        ):
            # --- PE warmup: dependency-free junk matmuls lift the HAM clock
            # gate while the first loads stream in.
            warm_sb = warmp.tile([P, SLAB_MAX], bf16, name="warm_sb",
                                 tag="warm_sb")
            nc.vector.memset(warm_sb[:], 0.0)
            warm_ps = warmpp.tile([P, SLAB_MAX], f32, name="warm_ps",
                                  tag="warm_ps")
            for i in range(N_WARM_LONG):
                nc.tensor.matmul(
                    warm_ps[:], warm_sb[:, 0:P], warm_sb[:],
                    start=(i == 0), stop=(i == N_WARM_LONG - 1),
                )
            for i in range(N_WARM_SHORT):
                nc.tensor.matmul(
                    warm_ps[:, 0:P], warm_sb[:, 0:P], warm_sb[:, 0:P],
                    start=True, stop=True,
                )

            # --- SBUF tiles: one big W tile so paired fb loads are single
            # DMAs with 4KB per-partition lines.
            w_sb = wp.tile([P, FB_N * IN_F], bf16, name="w", tag="w")
            x_sb = [None] * KC_N
            for kc in range(KC_N):
                x_sb[kc] = xp.tile([P, c_pad], bf16, name=f"x{kc}",
                                   tag=f"x{kc}")
            b_sb = bp.tile([P, FB_N], f32, name="bias", tag="bias")
            nc.scalar.dma_start(b_sb[:], b_r[:, :])

            def load_w(eng, fb_lo, fb_hi):
                eng.dma_start(
                    w_sb[:, fb_lo * IN_F:fb_hi * IN_F],
                    w_r[:, fb_lo * IN_F:fb_hi * IN_F],
                )

            def load_x(eng, kc):
                eng.dma_start(x_sb[kc][:], x_r[:, kc * c_pad:(kc + 1) * c_pad])

            # consumption-deadline order, split across both HWDGE rings
            load_w(nc.sync, 0, 1)
            load_w(nc.scalar, 1, 2)
            for kc in range(KC_N):
                load_x(nc.sync if kc % 2 == 0 else nc.scalar, kc)
            load_w(nc.sync, 2, 4)
            load_w(nc.scalar, 4, 6)
            load_w(nc.sync, 6, 8)
            load_w(nc.scalar, 8, 10)
            load_w(nc.sync, 10, 12)
            load_w(nc.scalar, 12, 14)
            load_w(nc.sync, 14, 16)

            # --- compute + evict + store
            def mm(psum, fb, kc, r0, s0, s1):
                nc.tensor.matmul(
                    psum[:, 0:s1 - s0],
                    w_sb[:, fb * IN_F + kc * P:fb * IN_F + (kc + 1) * P],
                    x_sb[kc][:, r0 + s0:r0 + s1],
                    start=(kc == 0),
                    stop=(kc == KC_N - 1),
                )

            ev_flip = [0]

            def evict(ot, psum, fb, s0, s1):
                # alternate DVE / ACT so neither engine gates the tail
                if ev_flip[0] % 2 == 0:
                    nc.vector.tensor_scalar_add(
                        ot[:, s0:s1], psum[:, 0:s1 - s0], b_sb[:, fb:fb + 1]
                    )
                else:
                    nc.scalar.activation(
                        ot[:, s0:s1], psum[:, 0:s1 - s0],
                        mybir.ActivationFunctionType.Identity,
                        bias=b_sb[:, fb:fb + 1], scale=1.0,
                    )
                ev_flip[0] += 1

            for ci, (r0, sizes) in enumerate(chunks):
                last_chunk = ci == len(chunks) - 1
                L = sum(sizes)
                bounds = []
                pos = 0
                for s in sizes:
                    bounds.append((pos, pos + s))
                    pos += s

                if ci == 0:
                    # fb0+fb1 kc-interleaved, paced by x-chunk arrival
                    ps = {}
                    for fb in (0, 1):
                        for si in range(len(sizes)):
                            ps[fb, si] = pp.tile(
                                [P, SLAB_MAX], f32,
                                name=f"ps_c0_{fb}_{si}", tag="psum",
                            )
                    for kc in range(KC_N):
                        for fb in (0, 1):
                            for si, (s0, s1) in enumerate(bounds):
                                mm(ps[fb, si], fb, kc, r0, s0, s1)
                    for fb in (0, 1):
                        ot = op.tile([P, CHUNK_MAX], bf16,
                                     name=f"ot_c0_{fb}", tag="ot")
                        for si, (s0, s1) in enumerate(bounds):
                            evict(ot, ps[fb, si], fb, s0, s1)
                        nc.scalar.dma_start(
                            out[fb * P:(fb + 1) * P, r0:r0 + L], ot[:, 0:L]
                        )
                    fb_start = 2
                else:
                    fb_start = 0

                # remaining fbs run slab-major: each slab's psum is evicted
                # (and, for the last fb, stored) while the next slab computes
                for fb in range(fb_start, FB_N):
                    last_fb = last_chunk and fb == FB_N - 1
                    ot = op.tile([P, CHUNK_MAX], bf16,
                                 name=f"ot_c{ci}_{fb}", tag="ot")
                    for si, (s0, s1) in enumerate(bounds):
                        psum = pp.tile([P, SLAB_MAX], f32,
                                       name=f"ps_c{ci}_{fb}_{si}", tag="psum")
                        for kc in range(KC_N):
                            mm(psum, fb, kc, r0, s0, s1)
                        evict(ot, psum, fb, s0, s1)
                        if last_fb:
                            # final stores ride the (idle by now) sync ring,
                            # one per slab, right behind each evict
                            nc.sync.dma_start(
                                out[fb * P:(fb + 1) * P, r0 + s0:r0 + s1],
                                ot[:, s0:s1],
                            )
                    if not last_fb:
                        nc.scalar.dma_start(
                            out[fb * P:(fb + 1) * P, r0:r0 + L], ot[:, 0:L]
                        )

    nc.compile()
    return nc


def _get_nc(c_pad: int):
    nc = _nc_cache.get(c_pad)
    if nc is None:
        nc = _build_nc(c_pad)
        _nc_cache[c_pad] = nc
    return nc


def kernel(x, indices, W, b):
    global LAST_EXEC_NS, LAST_RESULTS
    import ml_dtypes

    bf16 = np.dtype(ml_dtypes.bfloat16)

    x = np.ascontiguousarray(np.asarray(x, dtype=np.float32))
    W = np.ascontiguousarray(np.asarray(W, dtype=np.float32))
    b = np.asarray(b, dtype=np.float32)
    idx = np.asarray(indices).astype(np.int64)

    order = np.argsort(idx, kind="stable")
    counts = np.bincount(idx, minlength=G)
    offs = np.zeros(G + 1, dtype=np.int64)
    np.cumsum(counts, out=offs[1:])

    c_pad = max(P, int(-(-int(counts.max()) // 8)) * 8)
    nc = _get_nc(c_pad)

    rows = [order[offs[g]:offs[g + 1]] for g in range(G)]
    in_maps = []
    for g in range(G):
        cg = int(counts[g])
        # x_r [128, kc*c_pad + r] = x[rows[r], kc*128 + p]
        xT = np.zeros((IN_F, c_pad), dtype=np.float32)
        if cg:
            xT[:, :cg] = x[rows[g]].T
        xr = np.ascontiguousarray(
            xT.reshape(KC_N, P, c_pad).transpose(1, 0, 2).reshape(P, KC_N * c_pad)
        ).astype(bf16)
        # w_r [128, fb*1024 + kc*128 + f] = W_g[fb*128+f, kc*128+p]
        wT = W[g * OUT_F:(g + 1) * OUT_F, :].T  # [1024(k), 2048(f)]
        wr = np.ascontiguousarray(
            wT.reshape(KC_N, P, FB_N, P).transpose(1, 2, 0, 3).reshape(P, FB_N * IN_F)
        ).astype(bf16)
        # b_r [128, fb] = b_g[fb*128 + p]
        br = np.ascontiguousarray(
            b[g * OUT_F:(g + 1) * OUT_F].reshape(FB_N, P).T
        ).astype(np.float32)
        in_maps.append({"x_r": xr, "w_r": wr, "b_r": br})

    trace = bool(int(os.environ.get("KERNEL_TRACE", "0")))
    res = run_bass_kernel_spmd(nc, in_maps, list(range(NCORES)), trace=trace)
    LAST_EXEC_NS = res.exec_time_ns
    LAST_RESULTS = res

    out = np.empty((N, OUT_F), dtype=np.float32)
    for g in range(G):
        cg = int(counts[g])
        if cg:
            out[rows[g]] = res.results[g]["out"][:, :cg].T.astype(np.float32)
    return out


# revision 5
# speedup vs baseline: 1.0463x; 1.0463x over previous
"""GroupLinear (MoE routing) Trainium2 kernel — bf16 W-stationary version.

Problem: x [8192, 1024] f32, indices [8192] int64 in [0,8),
W [8*2048, 1024] f32, b [8*2048] f32.
out[n] = x[n] @ W[g*2048:(g+1)*2048].T + b[g*2048:(g+1)*2048],  g = indices[n].

Strategy: expert-parallel across the 8 NeuronCores. Core g owns group g's
weight slice only and processes exactly the rows routed to group g (argsort
on host). Operands are cast to bf16 on host (rel-err ~3e-3, full PE rate,
half the HBM traffic of f32).

Device layout (W-stationary): the PE's stationary operand is a [128k x 128f]
W chunk; x rows are the MOVING operand, so per-core PE time is exactly
128 * c_pad cycles (no padding of rows to 128-blocks). PSUM tiles are
[128f x slab<=512 rows]; eviction adds the per-feature bias (it varies along
partitions) and casts to bf16, alternating between DVE (tensor_scalar_add)
and ACT (activation Identity+bias) so no single engine gates the tail.
Output leaves the device transposed [2048f x c_pad rows]; host transposes
back (host time is not graded).

Schedule: loads are split across BOTH HWDGE rings so the startup ramp is
fed at combined rate: sync gets [w0, x0, x2, x4, x6, w23, w67, w10-11,
w14-15], scalar gets [bias, w1, x1, x3, x5, x7, w45, w89, w12-13] then the
stores. fb0+fb1 are computed kc-interleaved, paced by x-chunk arrival; the
remaining 14 feature blocks are compute-bound and run slab-major so each
PSUM slab is evicted + stored while the next one computes. The last fb's
slab stores ride the (by then idle) sync ring. A short junk-matmul burst
lifts the PE HAM clock gate while the first loads are in flight.
"""

import os
import sys

sys.path.insert(0, "/opt/trn_rl_repo")

import numpy as np

import concourse.bass as bass
import concourse.bacc as bacc
import concourse.mybir as mybir
import concourse.tile as tile
from concourse.bass_utils import run_bass_kernel_spmd

N = 8192
IN_F = 1024
OUT_F = 2048
G = 8
NCORES = 8
P = 128
KC_N = IN_F // P      # 8 contraction chunks
FB_N = OUT_F // P     # 16 feature blocks (stationary tiles per kc)
SLAB_MAX = 512        # PSUM bank free size in fp32
CHUNK_MAX = 1512      # rows per chunk -> <=3 slabs -> <=3 psum banks per fb

N_WARM_LONG = 5       # junk matmuls N=512 (cold ~427ns each)
N_WARM_SHORT = 6      # junk matmuls N=128 (cold ~107ns each)

LAST_EXEC_NS = None
LAST_RESULTS = None

_nc_cache = {}


def _chunk_plan(c_pad):
    """Split c_pad rows into chunks of <=3 slabs, slab sizes multiple of 8."""
    chunks = []
    r0 = 0
    while r0 < c_pad:
        rem = c_pad - r0
        L = rem if rem <= 3 * SLAB_MAX else CHUNK_MAX
        n_s = -(-L // SLAB_MAX)
        base = -(-(-(-L // n_s)) // 8) * 8
        sizes = [base] * (n_s - 1)
        sizes.append(L - base * (n_s - 1))
        chunks.append((r0, sizes))
        r0 += L
    return chunks


def _build_nc(c_pad: int):
    """Per-core Bass program for c_pad routed rows (same program, all cores)."""
    assert c_pad % 8 == 0
    chunks = _chunk_plan(c_pad)
    f32 = mybir.dt.float32
    bf16 = mybir.dt.bfloat16

    nc = bacc.Bacc("TRN2", target_bir_lowering=False, debug=False)

    x_r = nc.dram_tensor("x_r", [P, KC_N * c_pad], bf16, kind="ExternalInput")
    w_r = nc.dram_tensor("w_r", [P, FB_N * IN_F], bf16, kind="ExternalInput")
    b_r = nc.dram_tensor("b_r", [P, FB_N], f32, kind="ExternalInput")
    out = nc.dram_tensor("out", [OUT_F, c_pad], bf16, kind="ExternalOutput")

    with tile.TileContext(nc) as tc:
        with (
            tc.tile_pool(name="wp", bufs=1) as wp,
            tc.tile_pool(name="xp", bufs=1) as xp,
            tc.tile_pool(name="bp", bufs=1) as bp,
            tc.tile_pool(name="op", bufs=3) as op,
            tc.tile_pool(name="pp", bufs=7, space="PSUM") as pp,
            tc.tile_pool(name="warm", bufs=1) as warmp,
            tc.tile_pool(name="warmps", bufs=1, space="PSUM") as warmpp,
        ):
            # --- PE warmup: dependency-free junk matmuls lift the HAM clock
            # gate while the first loads stream in.
            warm_sb = warmp.tile([P, SLAB_MAX], bf16, name="warm_sb",
                                 tag="warm_sb")
            nc.vector.memset(warm_sb[:], 0.0)
            warm_ps = warmpp.tile([P, SLAB_MAX], f32, name="warm_ps",
                                  tag="warm_ps")
            for i in range(N_WARM_LONG):
                nc.tensor.matmul(
                    warm_ps[:], warm_sb[:, 0:P], warm_sb[:],
                    start=(i == 0), stop=(i == N_WARM_LONG - 1),
                )
            for i in range(N_WARM_SHORT):
                nc.tensor.matmul(
                    warm_ps[:, 0:P], warm_sb[:, 0:P], warm_sb[:, 0:P],
                    start=True, stop=True,
                )

            # --- SBUF tiles: one big W tile so paired fb loads are single
            # DMAs with 4KB per-partition lines.
            w_sb = wp.tile([P, FB_N * IN_F], bf16, name="w", tag="w")
            x_sb = [None] * KC_N
            for kc in range(KC_N):
                x_sb[kc] = xp.tile([P, c_pad], bf16, name=f"x{kc}",
                                   tag=f"x{kc}")
            b_sb = bp.tile([P, FB_N], f32, name="bias", tag="bias")
            nc.scalar.dma_start(b_sb[:], b_r[:, :])

            def load_w(eng, fb_lo, fb_hi):
                eng.dma_start(
                    w_sb[:, fb_lo * IN_F:fb_hi * IN_F],
                    w_r[:, fb_lo * IN_F:fb_hi * IN_F],
                )

            def load_x(eng, kc):
                eng.dma_start(x_sb[kc][:], x_r[:, kc * c_pad:(kc + 1) * c_pad])

            # consumption-deadline order, all on the sync ring (the two
            # HWDGE rings share the 16 SDMA engines, so splitting loads
            # across rings does not add bandwidth — it only reorders)
            load_w(nc.sync, 0, 1)
            load_x(nc.sync, 0)
            load_w(nc.sync, 1, 2)
            for kc in range(1, KC_N):
                load_x(nc.sync, kc)
            for fb in range(2, FB_N, 2):
                load_w(nc.sync, fb, fb + 2)

            # --- compute + evict + store
            def mm(psum, fb, kc, r0, s0, s1):
                nc.tensor.matmul(
                    psum[:, 0:s1 - s0],
                    w_sb[:, fb * IN_F + kc * P:fb * IN_F + (kc + 1) * P],
                    x_sb[kc][:, r0 + s0:r0 + s1],
                    start=(kc == 0),
                    stop=(kc == KC_N - 1),
                )

            ev_flip = [0]

            def evict(ot, psum, fb, s0, s1):
                # alternate DVE / ACT so neither engine gates the tail
                if ev_flip[0] % 2 == 0:
                    nc.vector.tensor_scalar_add(
                        ot[:, s0:s1], psum[:, 0:s1 - s0], b_sb[:, fb:fb + 1]
                    )
                else:
                    nc.scalar.activation(
                        ot[:, s0:s1], psum[:, 0:s1 - s0],
                        mybir.ActivationFunctionType.Identity,
                        bias=b_sb[:, fb:fb + 1], scale=1.0,
                    )
                ev_flip[0] += 1

            for ci, (r0, sizes) in enumerate(chunks):
                last_chunk = ci == len(chunks) - 1
                L = sum(sizes)
                bounds = []
                pos = 0
                for s in sizes:
                    bounds.append((pos, pos + s))
                    pos += s

                if ci == 0:
                    # fb0+fb1 kc-interleaved, paced by x-chunk arrival
                    ps = {}
                    for fb in (0, 1):
                        for si in range(len(sizes)):
                            ps[fb, si] = pp.tile(
                                [P, SLAB_MAX], f32,
                                name=f"ps_c0_{fb}_{si}", tag="psum",
                            )
                    for kc in range(KC_N):
                        for fb in (0, 1):
                            for si, (s0, s1) in enumerate(bounds):
                                mm(ps[fb, si], fb, kc, r0, s0, s1)
                    for fb in (0, 1):
                        ot = op.tile([P, CHUNK_MAX], bf16,
                                     name=f"ot_c0_{fb}", tag="ot")
                        for si, (s0, s1) in enumerate(bounds):
                            evict(ot, ps[fb, si], fb, s0, s1)
                        nc.scalar.dma_start(
                            out[fb * P:(fb + 1) * P, r0:r0 + L], ot[:, 0:L]
                        )
                    fb_start = 2
                else:
                    fb_start = 0

                # remaining fbs run slab-major: each slab's psum is evicted
                # (and, for the last fb, stored) while the next slab computes
                for fb in range(fb_start, FB_N):
                    last_fb = last_chunk and fb == FB_N - 1
                    ot = op.tile([P, CHUNK_MAX], bf16,
                                 name=f"ot_c{ci}_{fb}", tag="ot")
                    for si, (s0, s1) in enumerate(bounds):
                        psum = pp.tile([P, SLAB_MAX], f32,
                                       name=f"ps_c{ci}_{fb}_{si}", tag="psum")
                        for kc in range(KC_N):
                            mm(psum, fb, kc, r0, s0, s1)
                        evict(ot, psum, fb, s0, s1)
                        if last_fb:
                            # final stores ride the (idle by now) sync ring,
                            # one per slab, right behind each evict
                            nc.sync.dma_start(
                                out[fb * P:(fb + 1) * P, r0 + s0:r0 + s1],
                                ot[:, s0:s1],
                            )
                    if not last_fb:
                        nc.scalar.dma_start(
                            out[fb * P:(fb + 1) * P, r0:r0 + L], ot[:, 0:L]
                        )

    nc.compile()
    return nc


def _get_nc(c_pad: int):
    nc = _nc_cache.get(c_pad)
    if nc is None:
        nc = _build_nc(c_pad)
        _nc_cache[c_pad] = nc
    return nc


def kernel(x, indices, W, b):
    global LAST_EXEC_NS, LAST_RESULTS
    import ml_dtypes

    bf16 = np.dtype(ml_dtypes.bfloat16)

    x = np.ascontiguousarray(np.asarray(x, dtype=np.float32))
    W = np.ascontiguousarray(np.asarray(W, dtype=np.float32))
    b = np.asarray(b, dtype=np.float32)
    idx = np.asarray(indices).astype(np.int64)

    order = np.argsort(idx, kind="stable")
    counts = np.bincount(idx, minlength=G)
    offs = np.zeros(G + 1, dtype=np.int64)
    np.cumsum(counts, out=offs[1:])

    c_pad = max(P, int(-(-int(counts.max()) // 8)) * 8)
    nc = _get_nc(c_pad)

    rows = [order[offs[g]:offs[g + 1]] for g in range(G)]
    in_maps = []
    for g in range(G):
        cg = int(counts[g])
        # x_r [128, kc*c_pad + r] = x[rows[r], kc*128 + p]
        xT = np.zeros((IN_F, c_pad), dtype=np.float32)
        if cg:
            xT[:, :cg] = x[rows[g]].T
        xr = np.ascontiguousarray(
            xT.reshape(KC_N, P, c_pad).transpose(1, 0, 2).reshape(P, KC_N * c_pad)
        ).astype(bf16)
        # w_r [128, fb*1024 + kc*128 + f] = W_g[fb*128+f, kc*128+p]
        wT = W[g * OUT_F:(g + 1) * OUT_F, :].T  # [1024(k), 2048(f)]
        wr = np.ascontiguousarray(
            wT.reshape(KC_N, P, FB_N, P).transpose(1, 2, 0, 3).reshape(P, FB_N * IN_F)
        ).astype(bf16)
        # b_r [128, fb] = b_g[fb*128 + p]
        br = np.ascontiguousarray(
            b[g * OUT_F:(g + 1) * OUT_F].reshape(FB_N, P).T
        ).astype(np.float32)
        in_maps.append({"x_r": xr, "w_r": wr, "b_r": br})

    trace = bool(int(os.environ.get("KERNEL_TRACE", "0")))
    res = run_bass_kernel_spmd(nc, in_maps, list(range(NCORES)), trace=trace)
    LAST_EXEC_NS = res.exec_time_ns
    LAST_RESULTS = res

    out = np.empty((N, OUT_F), dtype=np.float32)
    for g in range(G):
        cg = int(counts[g])
        if cg:
            out[rows[g]] = res.results[g]["out"][:, :cg].T.astype(np.float32)
    return out
